# revision 3
# baseline (speedup 1.0000x reference)
"""Trainium2 Bass kernel for nn_EnhanceDiversityFeatureExtracition.

loss = mean((output - target)^2)
     + ALPHA * sum(G where TAU < G <= 1, off-diagonal)
  G  = cosine Gram of V[f] = conv_w[:, :, f, :].reshape(-1), f in [0, 128)

Device strategy (8 cores, SPMD, no collectives — host reduces):
 - conv_w viewed flat as [65536, 384] (row = (o, i), col = f*3 + k).
   Rows are sharded 8192/core. Each core accumulates the *flat-layout*
   384x384 Gram  G384[c1, c2] = sum_rows W[r, c1] * W[r, c2]  via
   PE matmuls in float32r (full-rate fp32 matmul at N>=256; ample
   precision vs. the 0.035 margin to the TAU threshold).  By symmetry
   only the blocks with c1<=127 (full width) and the lower-right
   [128:384] x [128:384] part are computed; the host mirrors the rest.
   The true filter Gram is the per-k diagonal
   S[f1, f2] = sum_k G384[3 f1 + k, 3 f2 + k], extracted on host.
 - output/target sharded 1024 rows/core; DVE computes d = a - b and a
   fused (d*1)*d with per-partition accumulate => MSE partial sums.
Host combines partials in float64 and returns the f32 scalar loss.

Schedule (v3): big 1.57 MB conv transfers with MSE transfers woven
between them (each MSE transfer is a PE catch-up window), and tiny
conv tiles at the very end so the post-stream tail is ~3 matmuls plus
pipelined PSUM->SBUF copies and per-slice output DMAs.
"""

import numpy as np

ALPHA = 0.0005
TAU = 0.2

P = 128
NCORES = 8

# conv_w [256, 256, 128, 3] -> flat [65536, 384]
W_ROWS = 65536
W_COLS = 384
W_ROWS_PER_CORE = W_ROWS // NCORES  # 8192 = 64 chunks of 128
W_JS = [8] * 7 + [4, 2, 1, 1]  # rows-per-partition per conv tile (sum 64)
# Gram slices: (lhsT col range, rhs col range)
G_SLICES = [(0, 0, 384), (128, 128, 256), (256, 128, 256)]
G_OUT = 384 + 256 + 256  # 896 columns in the packed gout

# output/target [8192, 1000]
B_ROWS = 8192
B_COLS = 1000
B_ROWS_PER_CORE = B_ROWS // NCORES  # 1024
M_TILES = 4
M_J = B_ROWS_PER_CORE // (M_TILES * P)  # 2 rows/partition per tile

_CACHE = {}
LAST_RESULTS = None  # BassKernelResults of the most recent run (for test.py)


def _build_nc():
    import concourse.tile as tile
    from concourse import bacc, mybir

    nc = bacc.Bacc("TRN2", target_bir_lowering=False, debug=False,
                   num_devices=NCORES)
    f32 = mybir.dt.float32
    f32r = mybir.dt.float32r

    wsh = nc.dram_tensor("wsh", [W_ROWS_PER_CORE, W_COLS], f32r,
                         kind="ExternalInput").ap()
    osh = nc.dram_tensor("osh", [B_ROWS_PER_CORE, B_COLS], f32,
                         kind="ExternalInput").ap()
    tsh = nc.dram_tensor("tsh", [B_ROWS_PER_CORE, B_COLS], f32,
                         kind="ExternalInput").ap()
    gout = nc.dram_tensor("gout", [P, G_OUT], f32,
                          kind="ExternalOutput").ap()
    mout = nc.dram_tensor("mout", [P, M_TILES], f32,
                          kind="ExternalOutput").ap()

    ov = osh.rearrange("(t p j) f -> t p j f", t=M_TILES, p=P)
    tv = tsh.rearrange("(t p j) f -> t p j f", t=M_TILES, p=P)

    n_chunks = W_ROWS_PER_CORE // P  # 64 accumulating matmuls per psum tile

    with tile.TileContext(nc) as tc:
        with (
            tc.tile_pool(name="wpool", bufs=1) as wpool,
            tc.tile_pool(name="apool", bufs=3) as apool,
            tc.tile_pool(name="bpool", bufs=3) as bpool,
            tc.tile_pool(name="dpool", bufs=2) as dpool,
            tc.tile_pool(name="acc", bufs=1) as acc,
            tc.tile_pool(name="psum", bufs=1, space="PSUM") as psum,
        ):
            g_ps = [
                psum.tile([P, n], f32, name=f"g{m}", tag=f"g{m}")
                for m, (_, _, n) in enumerate(G_SLICES)
            ]
            mse_cols = acc.tile([P, M_TILES], f32, name="mse_cols")
            gs = acc.tile([P, G_OUT], f32, name="gs")

            def load_mse(t, which):
                if which == 0:
                    at = apool.tile([P, M_J, B_COLS], f32, name="at",
                                    tag="at")
                    nc.sync.dma_start(at[:], ov[t])
                    return at
                bt = bpool.tile([P, M_J, B_COLS], f32, name="bt", tag="bt")
                nc.sync.dma_start(bt[:], tv[t])
                return bt

            def mse_compute(t, at, bt):
                d = dpool.tile([P, M_J, B_COLS], f32, name="d", tag="d")
                nc.vector.tensor_tensor(d[:], at[:], bt[:],
                                        mybir.AluOpType.subtract)
                d2 = dpool.tile([P, M_J, B_COLS], f32, name="d2", tag="d2")
                nc.vector.scalar_tensor_tensor(
                    d2[:], d[:], 1.0, d[:],
                    op0=mybir.AluOpType.mult, op1=mybir.AluOpType.mult,
                    accum_out=mse_cols[:, t:t + 1],
                )

            # DMA stream order: w tile, then one MSE transfer between
            # conv tiles.  MSE transfer i of (a0,b0,a1,b1,...) follows
            # w tile i; m3's pair lands before the last big conv tile.
            mse_pending = {}
            chunk = 0
            row0 = 0
            for t, wj in enumerate(W_JS):
                wt = wpool.tile([P, wj, W_COLS], f32r, name="wt",
                                tag=f"wt{wj}", bufs=(4 if wj == 8 else 2))
                src = wsh[row0:row0 + P * wj].rearrange(
                    "(p j) c -> p j c", j=wj)
                nc.sync.dma_start(wt[:], src)
                row0 += P * wj
                for j in range(wj):
                    is_last = (chunk == n_chunks - 1)
                    for m, (lh0, rh0, n) in enumerate(G_SLICES):
                        nc.tensor.matmul(
                            g_ps[m][:],
                            wt[:, j, lh0:lh0 + P],
                            wt[:, j, rh0:rh0 + n],
                            start=(chunk == 0),
                            stop=is_last,
                        )
                    chunk += 1
                if is_last:
                    col = 0
                    for m, (_, _, n) in enumerate(G_SLICES):
                        nc.vector.tensor_copy(gs[:, col:col + n], g_ps[m][:])
                        nc.sync.dma_start(gout[:, col:col + n],
                                          gs[:, col:col + n])
                        col += n
                # one MSE transfer after each of the first 8 conv tiles
                if t < 8:
                    mt, which = t // 2, t % 2
                    mse_pending[(mt, which)] = load_mse(mt, which)
                    if which == 1:
                        mse_compute(mt, mse_pending.pop((mt, 0)),
                                    mse_pending.pop((mt, 1)))
                    if mt == 3 and which == 1:
                        nc.sync.dma_start(mout[:], mse_cols[:])

    nc.compile()
    return nc


def kernel(output, target, conv_w):
    global LAST_RESULTS
    from concourse.bass_utils import run_bass_kernel_spmd

    output = np.ascontiguousarray(np.asarray(output, dtype=np.float32))
    target = np.ascontiguousarray(np.asarray(target, dtype=np.float32))
    conv_w = np.ascontiguousarray(np.asarray(conv_w, dtype=np.float32))
    assert output.shape == (B_ROWS, B_COLS)
    assert target.shape == (B_ROWS, B_COLS)
    assert conv_w.shape == (256, 256, 128, 3)

    if "nc" not in _CACHE:
        _CACHE["nc"] = _build_nc()
    nc = _CACHE["nc"]

    w_flat = conv_w.reshape(W_ROWS, W_COLS)
    in_maps = []
    for c in range(NCORES):
        in_maps.append({
            "wsh": w_flat[c * W_ROWS_PER_CORE:(c + 1) * W_ROWS_PER_CORE],
            "osh": output[c * B_ROWS_PER_CORE:(c + 1) * B_ROWS_PER_CORE],
            "tsh": target[c * B_ROWS_PER_CORE:(c + 1) * B_ROWS_PER_CORE],
        })

    res = run_bass_kernel_spmd(nc, in_maps, core_ids=list(range(NCORES)))
    LAST_RESULTS = res

    # ---- host reduction (tiny) ----
    g = np.zeros((P, G_OUT), dtype=np.float64)
    mse_sum = 0.0
    for r in res.results:
        g += r["gout"].astype(np.float64)
        mse_sum += float(r["mout"].astype(np.float64).sum())

    # assemble G384 from the computed blocks + symmetry
    g384 = np.zeros((W_COLS, W_COLS), dtype=np.float64)
    g384[0:128, :] = g[:, 0:384]                 # rows 0:128, all cols
    g384[128:256, 128:384] = g[:, 384:640]       # (1,1) (1,2)
    g384[256:384, 128:384] = g[:, 640:896]       # (2,1) (2,2)
    g384[128:384, 0:128] = g384[0:128, 128:384].T  # (1,0) (2,0)

    # S[f1, f2] = sum_k G384[3 f1 + k, 3 f2 + k]
    s = np.einsum("ikjk->ij", g384.reshape(P, 3, P, 3))
    norms = np.sqrt(np.diag(s))
    gcos = s / np.outer(norms, norms)
    offdiag = ~np.eye(P, dtype=bool)
    mask = (gcos > TAU) & (gcos <= 1.0) & offdiag
    reg = gcos[mask].sum()

    mse = mse_sum / (B_ROWS * B_COLS)
    return np.array(mse + ALPHA * reg, dtype=np.float32)


# revision 5
# speedup vs baseline: 1.0536x; 1.0536x over previous
"""Trainium2 Bass kernel for nn_EnhanceDiversityFeatureExtracition.

loss = mean((output - target)^2)
     + ALPHA * sum(G where TAU < G <= 1, off-diagonal)
  G  = cosine Gram of V[f] = conv_w[:, :, f, :].reshape(-1), f in [0, 128)

Device strategy (8 cores, SPMD, no collectives — host reduces):
 - conv_w viewed flat as [65536, 384] (row = (o, i), col = f*3 + k).
   Rows are sharded 8192/core. Each core accumulates the *flat-layout*
   384x384 Gram  G384[c1, c2] = sum_rows W[r, c1] * W[r, c2]  via
   PE matmuls in float32r (full-rate fp32 matmul at N>=256; ample
   precision vs. the 0.035 margin to the TAU threshold).  By symmetry
   only rows 0:128 (full width) and the [128:384] x [128:384] part are
   computed; the host mirrors the rest.  The true filter Gram is the
   per-k diagonal S[f1, f2] = sum_k G384[3 f1 + k, 3 f2 + k] (host).
 - output/target sharded 1024 rows/core; DVE computes d = a - b and a
   fused (d*1)*d with per-partition accumulate => MSE partial sums.
Host combines partials in float64 and returns the f32 scalar loss.

Schedule (v4): the entire conv_w stream goes first so the PE runs one
continuous warm matmul burst at DMA line rate; the MSE stream follows
(PE drains its small lag during it, DVE chains overlap the stream and
the last MSE pair is half-sized to shorten the tail).  Gram/MSE output
DMAs ride the GpSimd SWDGE queue so they never wait behind the input
stream on the Sync FIFO.
"""

import numpy as np

ALPHA = 0.0005
TAU = 0.2

P = 128
NCORES = 8

# conv_w [256, 256, 128, 3] -> flat [65536, 384]
W_ROWS = 65536
W_COLS = 384
W_ROWS_PER_CORE = W_ROWS // NCORES  # 8192 = 64 chunks of 128
W_JS = [4, 8, 8, 8, 8, 8, 8, 8, 2, 1, 1]  # rows/partition per conv tile (sum 64)
# Gram slices: (lhsT col base, rhs col base, rhs width)
G_SLICES = [(0, 0, 384), (128, 128, 256), (256, 128, 256)]
G_OUT = 384 + 256 + 256  # 896 columns in the packed gout

# output/target [8192, 1000]
B_ROWS = 8192
B_COLS = 1000
B_ROWS_PER_CORE = B_ROWS // NCORES  # 1024
M_JS = [2, 2, 2, 1, 1]  # rows/partition per MSE tile (sum 8 -> 1024 rows)

_CACHE = {}
LAST_RESULTS = None  # BassKernelResults of the most recent run (for test.py)


def _build_nc():
    import concourse.tile as tile
    from concourse import bacc, mybir

    nc = bacc.Bacc("TRN2", target_bir_lowering=False, debug=False,
                   num_devices=NCORES)
    f32 = mybir.dt.float32
    f32r = mybir.dt.float32r

    wsh = nc.dram_tensor("wsh", [W_ROWS_PER_CORE, W_COLS], f32r,
                         kind="ExternalInput").ap()
    osh = nc.dram_tensor("osh", [B_ROWS_PER_CORE, B_COLS], f32,
                         kind="ExternalInput").ap()
    tsh = nc.dram_tensor("tsh", [B_ROWS_PER_CORE, B_COLS], f32,
                         kind="ExternalInput").ap()
    gout = nc.dram_tensor("gout", [P, G_OUT], f32,
                          kind="ExternalOutput").ap()
    mout = nc.dram_tensor("mout", [P, len(M_JS)], f32,
                          kind="ExternalOutput").ap()

    n_chunks = W_ROWS_PER_CORE // P  # 64 accumulating matmuls per psum tile

    with tile.TileContext(nc) as tc:
        with (
            tc.tile_pool(name="wpool", bufs=1) as wpool,
            tc.tile_pool(name="apool", bufs=3) as apool,
            tc.tile_pool(name="bpool", bufs=3) as bpool,
            tc.tile_pool(name="dpool", bufs=2) as dpool,
            tc.tile_pool(name="acc", bufs=1) as acc,
            tc.tile_pool(name="psum", bufs=1, space="PSUM") as psum,
        ):
            g_ps = [
                psum.tile([P, n], f32, name=f"g{m}", tag=f"g{m}")
                for m, (_, _, n) in enumerate(G_SLICES)
            ]
            mse_cols = acc.tile([P, len(M_JS)], f32, name="mse_cols")
            gs = acc.tile([P, G_OUT], f32, name="gs")

            # ---- conv_w stream: one continuous warm PE burst ----
            chunk = 0
            row0 = 0
            for t, wj in enumerate(W_JS):
                wt = wpool.tile([P, wj, W_COLS], f32r, name="wt",
                                tag=f"wt{wj}",
                                bufs=(3 if wj == 8 else 1))
                src = wsh[row0:row0 + P * wj].rearrange(
                    "(p j) c -> p j c", j=wj)
                nc.sync.dma_start(wt[:], src)
                row0 += P * wj
                for j in range(wj):
                    is_last = (chunk == n_chunks - 1)
                    for m, (lh0, rh0, n) in enumerate(G_SLICES):
                        nc.tensor.matmul(
                            g_ps[m][:],
                            wt[:, j, lh0:lh0 + P],
                            wt[:, j, rh0:rh0 + n],
                            start=(chunk == 0),
                            stop=is_last,
                        )
                    chunk += 1
                if is_last:
                    col = 0
                    for m, (_, _, n) in enumerate(G_SLICES):
                        nc.vector.tensor_copy(gs[:, col:col + n], g_ps[m][:])
                        # SWDGE queue: doesn't wait behind the Sync FIFO
                        nc.gpsimd.dma_start(gout[:, col:col + n],
                                            gs[:, col:col + n])
                        col += n

            # ---- MSE stream ----
            brow = 0
            for t, mj in enumerate(M_JS):
                at = apool.tile([P, 2, B_COLS], f32, name="at",
                                tag="at")[:, :mj, :]
                bt = bpool.tile([P, 2, B_COLS], f32, name="bt",
                                tag="bt")[:, :mj, :]
                osrc = osh[brow:brow + P * mj].rearrange(
                    "(p j) f -> p j f", j=mj)
                tsrc = tsh[brow:brow + P * mj].rearrange(
                    "(p j) f -> p j f", j=mj)
                nc.sync.dma_start(at[:], osrc)
                nc.sync.dma_start(bt[:], tsrc)
                brow += P * mj
                d = dpool.tile([P, 2, B_COLS], f32, name="d",
                               tag="d")[:, :mj, :]
                nc.vector.tensor_tensor(d[:], at[:], bt[:],
                                        mybir.AluOpType.subtract)
                d2 = dpool.tile([P, 2, B_COLS], f32, name="d2",
                                tag="d2")[:, :mj, :]
                nc.vector.scalar_tensor_tensor(
                    d2[:], d[:], 1.0, d[:],
                    op0=mybir.AluOpType.mult, op1=mybir.AluOpType.mult,
                    accum_out=mse_cols[:, t:t + 1],
                )
            nc.gpsimd.dma_start(mout[:], mse_cols[:])

    nc.compile()
    return nc


def kernel(output, target, conv_w):
    global LAST_RESULTS
    from concourse.bass_utils import run_bass_kernel_spmd

    output = np.ascontiguousarray(np.asarray(output, dtype=np.float32))
    target = np.ascontiguousarray(np.asarray(target, dtype=np.float32))
    conv_w = np.ascontiguousarray(np.asarray(conv_w, dtype=np.float32))
    assert output.shape == (B_ROWS, B_COLS)
    assert target.shape == (B_ROWS, B_COLS)
    assert conv_w.shape == (256, 256, 128, 3)

    if "nc" not in _CACHE:
        _CACHE["nc"] = _build_nc()
    nc = _CACHE["nc"]

    w_flat = conv_w.reshape(W_ROWS, W_COLS)
    in_maps = []
    for c in range(NCORES):
        in_maps.append({
            "wsh": w_flat[c * W_ROWS_PER_CORE:(c + 1) * W_ROWS_PER_CORE],
            "osh": output[c * B_ROWS_PER_CORE:(c + 1) * B_ROWS_PER_CORE],
            "tsh": target[c * B_ROWS_PER_CORE:(c + 1) * B_ROWS_PER_CORE],
        })

    res = run_bass_kernel_spmd(nc, in_maps, core_ids=list(range(NCORES)))
    LAST_RESULTS = res

    # ---- host reduction (tiny) ----
    g = np.zeros((P, G_OUT), dtype=np.float64)
    mse_sum = 0.0
    for r in res.results:
        g += r["gout"].astype(np.float64)
        mse_sum += float(r["mout"].astype(np.float64).sum())

    # assemble G384 from the computed blocks + symmetry
    g384 = np.zeros((W_COLS, W_COLS), dtype=np.float64)
    g384[0:128, :] = g[:, 0:384]                   # rows 0:128, all cols
    g384[128:256, 128:384] = g[:, 384:640]         # (1,1) (1,2)
    g384[256:384, 128:384] = g[:, 640:896]         # (2,1) (2,2)
    g384[128:384, 0:128] = g384[0:128, 128:384].T  # (1,0) (2,0)

    # S[f1, f2] = sum_k G384[3 f1 + k, 3 f2 + k]
    s = np.einsum("ikjk->ij", g384.reshape(P, 3, P, 3))
    norms = np.sqrt(np.diag(s))
    gcos = s / np.outer(norms, norms)
    offdiag = ~np.eye(P, dtype=bool)
    mask = (gcos > TAU) & (gcos <= 1.0) & offdiag
    reg = gcos[mask].sum()

    mse = mse_sum / (B_ROWS * B_COLS)
    return np.array(mse + ALPHA * reg, dtype=np.float32)
